# revision 4
# baseline (speedup 1.0000x reference)
"""Triangular GEMM C = triu(A)@triu(B), N=4096 fp32, 8 trn2 cores, T=128.

Baseline ladder decomposition (126 chains, 748 matmuls/core, 128-wide
moving) upgraded with PAIRED ladders: adjacent output rows of the col-L16,
col-L8 and row-L8 ladders merge into one chain with 256-wide moving
operands (two A-rows / two B-cols packed side by side), halving the matmul
count for ~92% of the work. Stationary per k is shared by the pair by
construction. Per-core matmuls: 748 -> 432.

Uniform SPMD program; per-core behavior via host packing (A-cores 0-3,
B-cores 4-7 = transpose image, same as baseline).
"""

import numpy as np

N = 4096
T = 128
NB = N // T  # 32
P = 128
NCORES = 8

INPUT_DTYPE = "float16"
OUT_DTYPE = "float16"
PSUM_BUFS = 8
NPAN = 64
NSLOTS = 126
CH = 8
NSLOTS_PAD = 128
O_BUFS = 3

# ---- ex stack layout (row-L8 mov block removed vs baseline) ----
E_CL8S = 0      # 2x8  col-L8 stats
E_RL8S = 16     # 2x8  row-L8 stats (reversed)
E_CL4S0 = 32    # 4    col-L4 ss0 stats
E_CL4S1 = 36    # 4    col-L4 ss1 stats
E_RL4S0 = 40    # 4    row-L4 ss0 stats (reversed)
E_RL4M0 = 44    # 10   row-L4 ss0 movs
E_RL4S1 = 54    # 4    row-L4 ss1 stats (reversed)
E_RL4M1 = 58    # 10   row-L4 ss1 movs
E_CL2 = 68      # 2x(2+3) col-L2 a/b stats+movs
E_RL2 = 78      # 2x(2+3) row-L2 a/b stats+movs
E_S2 = 88       # 2x6  s2 cleanup
NEX = 100
NEX_PAD = 104

# tri stack: only the 20 entries col-L4 movs need (triu[0,4) + triu[8,12))
_TRI_LIST = ([(r, s) for r in range(4) for s in range(r, 4)]
             + [(8 + r, 8 + s) for r in range(4) for s in range(r, 4)])
TRI_MAP = {rs: i for i, rs in enumerate(_TRI_LIST)}
NTRI = 20
NTRI_PAD = 24

# trip stack: paired A-rows (2p, 2p+1) x k, k >= 2p: entries [P, 2T]
_TRIP_LIST = [(p, k) for p in range(8) for k in range(2 * p, 16)]
TRIP_MAP = {pk: i for i, pk in enumerate(_TRIP_LIST)}
NTRIP = len(_TRIP_LIST)              # 72 entries = 144 lanes
NTRIP_LANES = 144

# exp stack: paired B-cols for row-L8: entry (r, cp): B(8+r, 8+2cp:8+2cp+2),
# left half zero when 2cp < r (below diagonal)
_EXP_LIST = [(r, cp) for cp in range(4) for r in range(2 * cp + 2)]
EXP_MAP = {rc: i for i, rc in enumerate(_EXP_LIST)}
NEXP = len(_EXP_LIST)                # 20 entries = 40 lanes
NEXP_LANES = 48                      # pad to 6 chunks


def tri4(r, c):
    return r * 4 - r * (r - 1) // 2 + (c - r)


def _build_template():
    """chains: units=[(sref, mref)], mref may be wide; out=first slot,
    nout in {1,2}, cross flag for row-L8 half/lane swap."""
    chains = []
    slot = [0]

    def add(units, nout=1, cross=False):
        chains.append(dict(units=units, out=slot[0], nout=nout, cross=cross))
        slot[0] += nout

    # 4 col-L16 ladders, paired: pair p covers out rows 2p, 2p+1
    for l in range(4):
        for p in range(8):
            units = [(("pan", l * 16 + 2 * p), ("tripL", TRIP_MAP[(p, 2 * p)]))]
            units += [(("pan", l * 16 + k), ("trip", TRIP_MAP[(p, k)]))
                      for k in range(2 * p + 1, 16)]
            add(units, nout=2)
    # 2 col-L8 ladders, paired (k <= 7)
    for l in range(2):
        for p in range(4):
            units = [(("ex", E_CL8S + l * 8 + 2 * p),
                      ("tripL", TRIP_MAP[(p, 2 * p)]))]
            units += [(("ex", E_CL8S + l * 8 + k), ("trip", TRIP_MAP[(p, k)]))
                      for k in range(2 * p + 1, 8)]
            add(units, nout=2)
    # 2 row-L8 ladders, paired: pair q covers out cols 15-2q, 14-2q
    # psum right half = col 15-2q (slot first), left half = 14-2q (second)
    for l in range(2):
        for q in range(4):
            cp = 3 - q
            units = [(("ex", E_RL8S + l * 8 + 2 * q),
                      ("expR", EXP_MAP[(7 - 2 * q, cp)]))]
            units += [(("ex", E_RL8S + l * 8 + 2 * q + u),
                       ("exp", EXP_MAP[(7 - 2 * q - u, cp)]))
                      for u in range(1, 8 - 2 * q)]
            add(units, nout=2, cross=True)
    # col-L4 ss0 / ss1 (unchanged, tri remapped)
    for c in range(4):
        add([(("ex", E_CL4S0 + c + u), ("tri", TRI_MAP[(c, c + u)]))
             for u in range(4 - c)])
    for c in range(4):
        add([(("ex", E_CL4S1 + c + u), ("tri", TRI_MAP[(8 + c, 8 + c + u)]))
             for u in range(4 - c)])
    # row-L4 ss0 / ss1
    for base_s, base_m in ((E_RL4S0, E_RL4M0), (E_RL4S1, E_RL4M1)):
        for c in range(4):
            add([(("ex", base_s + c + u),
                  ("ex", base_m + tri4(3 - c - u, 3 - c)))
                 for u in range(4 - c)])
    # col-L2 a/b
    for g in range(2):
        s = E_CL2 + g * 5
        add([(("ex", s + 0), ("ex", s + 2)), (("ex", s + 1), ("ex", s + 3))])
        add([(("ex", s + 1), ("ex", s + 4))])
    # row-L2 a/b
    for g in range(2):
        s = E_RL2 + g * 5
        add([(("ex", s + 0), ("ex", s + 2)), (("ex", s + 1), ("ex", s + 3))])
        add([(("ex", s + 1), ("ex", s + 4))])
    # s2 x2
    for g in range(2):
        s = E_S2 + g * 6
        add([(("ex", s + 0), ("ex", s + 3)), (("ex", s + 1), ("ex", s + 4))])
        add([(("ex", s + 0), ("ex", s + 5))])
        add([(("ex", s + 2), ("ex", s + 4))])
    assert slot[0] == NSLOTS, slot[0]
    return chains


TEMPLATE = _build_template()


def _acore_filling(c):
    pan, tri, ex, trip, exp = {}, {}, {}, {}, {}
    outs = [None] * NSLOTS

    Jl = [16 + 4 * c + l for l in range(4)]
    for l in range(4):
        for t in range(16):
            pan[l * 16 + t] = ("B", t, Jl[l])
    for (r, s), i in TRI_MAP.items():
        tri[i] = ("A", r, s)
    for (p, k), i in TRIP_MAP.items():
        trip[i] = (("A", 2 * p, k) if k >= 2 * p else None,
                   ("A", 2 * p + 1, k) if k >= 2 * p + 1 else None)
    for (r, cp), i in EXP_MAP.items():
        exp[i] = (("B", 8 + r, 8 + 2 * cp) if 2 * cp >= r else None,
                  ("B", 8 + r, 9 + 2 * cp))
    J8 = [8 + c, 12 + c]
    for l in range(2):
        for t in range(8):
            ex[E_CL8S + l * 8 + t] = ("B", t, J8[l])
    I8 = [2 * c, 2 * c + 1]
    for l in range(2):
        for t in range(8):
            ex[E_RL8S + l * 8 + t] = ("A", I8[l], 15 - t)
    J4a, J4b = 4 + c, 12 + c
    for t in range(4):
        ex[E_CL4S0 + t] = ("B", t, J4a)
        ex[E_CL4S1 + t] = ("B", 8 + t, J4b)
    I4a, I4b = c, 8 + c
    for t in range(4):
        ex[E_RL4S0 + t] = ("A", I4a, 7 - t)
        ex[E_RL4S1 + t] = ("A", I4b, 15 - t)
    for r in range(4):
        for s in range(r, 4):
            ex[E_RL4M0 + tri4(r, s)] = ("B", 4 + r, 4 + s)
            ex[E_RL4M1 + tri4(r, s)] = ("B", 12 + r, 12 + s)
    cl2 = [(4 * c, 4 * c + 2), (4 * ((c + 1) % 4), 4 * ((c + 1) % 4) + 3)]
    for g, (b, J) in enumerate(cl2):
        s = E_CL2 + g * 5
        ex[s + 0] = ("B", b, J)
        ex[s + 1] = ("B", b + 1, J)
        ex[s + 2] = ("A", b, b)
        ex[s + 3] = ("A", b, b + 1)
        ex[s + 4] = ("A", b + 1, b + 1)
    rl2 = [(4 * ((c + 2) % 4), 4 * ((c + 2) % 4)),
           (4 * ((c + 3) % 4), 4 * ((c + 3) % 4) + 1)]
    for g, (b, I) in enumerate(rl2):
        s = E_RL2 + g * 5
        ex[s + 0] = ("A", I, b + 3)
        ex[s + 1] = ("A", I, b + 2)
        ex[s + 2] = ("B", b + 3, b + 3)
        ex[s + 3] = ("B", b + 2, b + 3)
        ex[s + 4] = ("B", b + 2, b + 2)
    b2s = [4 * c, 4 * c + 2]
    for g, b in enumerate(b2s):
        s = E_S2 + g * 6
        ex[s + 0] = ("A", b, b)
        ex[s + 1] = ("A", b, b + 1)
        ex[s + 2] = ("A", b + 1, b + 1)
        ex[s + 3] = ("B", b, b + 1)
        ex[s + 4] = ("B", b + 1, b + 1)
        ex[s + 5] = ("B", b, b)

    # ---- out slots (identical to baseline) ----
    slot = 0
    for l in range(4):
        for cc in range(16):
            outs[slot] = (cc, Jl[l], True); slot += 1
    for l in range(2):
        for cc in range(8):
            outs[slot] = (cc, J8[l], True); slot += 1
    for l in range(2):
        for cc in range(8):
            outs[slot] = (I8[l], 15 - cc, False); slot += 1
    for cc in range(4):
        outs[slot] = (cc, J4a, True); slot += 1
    for cc in range(4):
        outs[slot] = (8 + cc, J4b, True); slot += 1
    for cc in range(4):
        outs[slot] = (I4a, 7 - cc, False); slot += 1
    for cc in range(4):
        outs[slot] = (I4b, 15 - cc, False); slot += 1
    for g, (b, J) in enumerate(cl2):
        outs[slot] = (b, J, True); slot += 1
        outs[slot] = (b + 1, J, True); slot += 1
    for g, (b, I) in enumerate(rl2):
        outs[slot] = (I, b + 3, False); slot += 1
        outs[slot] = (I, b + 2, False); slot += 1
    for g, b in enumerate(b2s):
        outs[slot] = (b, b + 1, False); slot += 1
        outs[slot] = (b, b, False); slot += 1
        outs[slot] = (b + 1, b + 1, False); slot += 1
    assert slot == NSLOTS
    return dict(pan=pan, tri=tri, ex=ex, trip=trip, exp=exp), outs


def _tblock(blk):
    if blk is None:
        return None
    mat, bi, bj = blk
    return (("B", 31 - bj, 31 - bi) if mat == "A" else ("A", 31 - bj, 31 - bi))


def _bcore_filling(c):
    fill, outs = _acore_filling(c - 4)
    tfill = {}
    for stack, mp in fill.items():
        tfill[stack] = {}
        for idx, v in mp.items():
            if stack in ("trip", "exp"):
                tfill[stack][idx] = (_tblock(v[0]), _tblock(v[1]))
            else:
                tfill[stack][idx] = _tblock(v)
    touts = [(31 - J, 31 - I, not tr) for (I, J, tr) in outs]
    return tfill, touts


_FILLINGS = [(_acore_filling(c) if c < 4 else _bcore_filling(c))
             for c in range(NCORES)]


def _check_cover():
    seen = {}
    for c in range(NCORES):
        fill, outs = _FILLINGS[c]

        def get(ref):
            stack, idx = ref[0], ref[1]
            return fill[stack][idx]

        for ch in TEMPLATE:
            slots = [outs[ch["out"] + i] for i in range(ch["nout"])]
            for (ss, si), (ms_, mi) in ch["units"]:
                sb = fill[ss][si]
                if ms_ in ("trip", "exp", "tripL", "expR"):
                    stack = "trip" if ms_.startswith("trip") else "exp"
                    pair = fill[stack][mi]
                    halves = ([pair[0]] if ms_ == "tripL" else
                              [pair[1]] if ms_ == "expR" else list(pair))
                    if ms_ == "tripL":
                        outsl = [slots[0]]
                    elif ms_ == "expR":
                        outsl = [slots[0]]
                    elif ch["cross"]:
                        outsl = [slots[1], slots[0]]
                    else:
                        outsl = slots
                else:
                    halves = [fill[ms_][mi]]
                    outsl = [slots[0]]
                for mb, (I, J, transposed) in zip(halves, outsl):
                    if mb is None:
                        continue
                    ab = sb if sb[0] == "A" else mb
                    bb = sb if sb[0] == "B" else mb
                    assert ab[0] == "A" and bb[0] == "B", (c, sb, mb)
                    assert ab[1] == I and bb[2] == J, (c, I, J, ab, bb)
                    K = ab[2]
                    assert bb[1] == K, (c, I, J, K, ab, bb)
                    assert I <= K <= J, (c, I, K, J)
                    key = (I, K, J)
                    assert key not in seen, (key, seen.get(key), c)
                    seen[key] = c
    want = {(i, k, j) for i in range(NB) for k in range(i, NB)
            for j in range(k, NB)}
    assert set(seen) == want, (len(seen), len(want))


_check_cover()

_PROGRAMS = {}


def _build_program(repeat=1):
    import contextlib
    import concourse.bacc as bacc
    import concourse.mybir as mybir
    from concourse.tile import TileContext

    dt_in = getattr(mybir.dt, INPUT_DTYPE)
    dt_out = getattr(mybir.dt, OUT_DTYPE)
    f32 = mybir.dt.float32
    nc = bacc.Bacc("TRN2", target_bir_lowering=False, debug=False,
                   num_devices=NCORES)
    pan_in = nc.dram_tensor("pan", [NPAN // CH, P, CH, T], dt_in,
                            kind="ExternalInput")
    tri_in = nc.dram_tensor("tri", [NTRI_PAD // CH, P, CH, T], dt_in,
                            kind="ExternalInput")
    ex_in = nc.dram_tensor("ex", [NEX_PAD // CH, P, CH, T], dt_in,
                           kind="ExternalInput")
    trip_in = nc.dram_tensor("trip", [NTRIP_LANES // CH, P, CH, T], dt_in,
                             kind="ExternalInput")
    exp_in = nc.dram_tensor("exp", [NEXP_LANES // CH, P, CH, T], dt_in,
                            kind="ExternalInput")
    c_out = nc.dram_tensor("out_stack", [NSLOTS_PAD // CH, P, CH, T], dt_out,
                           kind="ExternalOutput")

    nch = {"pan": NPAN // CH, "tri": NTRI_PAD // CH, "ex": NEX_PAD // CH,
           "trip": NTRIP_LANES // CH, "exp": NEXP_LANES // CH}
    srcs = {"pan": pan_in, "tri": tri_in, "ex": ex_in, "trip": trip_in,
            "exp": exp_in}

    with TileContext(nc) as tc:
        with (
            tc.tile_pool(name="pan_pool", bufs=2 * nch["pan"]) as pan_pool,
            tc.tile_pool(name="tri_pool", bufs=2 * nch["tri"]) as tri_pool,
            tc.tile_pool(name="ex_pool", bufs=2 * nch["ex"]) as ex_pool,
            tc.tile_pool(name="trip_pool", bufs=2 * nch["trip"]) as trip_pool,
            tc.tile_pool(name="exp_pool", bufs=2 * nch["exp"]) as exp_pool,
            tc.tile_pool(name="o_pool", bufs=O_BUFS) as o_pool,
            tc.tile_pool(name="psum", bufs=PSUM_BUFS, space="PSUM") as psum_pool,
        ):
            pools = {"pan": pan_pool, "tri": tri_pool, "ex": ex_pool,
                     "trip": trip_pool, "exp": exp_pool}
            loop_ctx = (tc.For_i(0, repeat, 1) if repeat > 1
                        else contextlib.nullcontext())
            with loop_ctx:
                chunks = {}

                def load(stack, cidx):
                    t_ = pools[stack].tile([P, CH, T], dt_in, tag=stack,
                                           name=f"{stack}_{cidx}")
                    nc.sync.dma_start(out=t_, in_=srcs[stack][cidx])
                    chunks[(stack, cidx)] = t_

                for t in range(2):
                    load("pan", t)
                for i in range(nch["trip"]):
                    load("trip", i)
                for t in range(2, nch["pan"]):
                    load("pan", t)
                for i in range(nch["exp"]):
                    load("exp", i)
                for i in range(nch["tri"]):
                    load("tri", i)
                for i in range(nch["ex"]):
                    load("ex", i)

                def ap(ref):
                    stack, idx = ref
                    if stack in ("trip", "exp", "tripL", "expR"):
                        base = "trip" if stack.startswith("trip") else "exp"
                        lane = 2 * idx
                        tile = chunks[(base, lane // CH)]
                        o = lane % CH
                        if stack == "tripL":
                            return tile[:, o, :]
                        if stack == "expR":
                            return tile[:, o + 1, :]
                        return tile[:, o:o + 2, :]
                    return chunks[(stack, idx // CH)][:, idx % CH, :]

                o_t = None
                lane = 0
                flushed = 0
                for ci, ch in enumerate(TEMPLATE):
                    units = ch["units"]
                    L = len(units)
                    wide = ch["nout"] == 2
                    ps = psum_pool.tile([P, 2 * T] if wide else [P, T], f32,
                                        tag="ps", name=f"ps_{ch['out']}")
                    for u, (sref, mref) in enumerate(units):
                        if mref[0] in ("tripL",):
                            out_ap = ps[:, 0:T]
                        elif mref[0] in ("expR",):
                            out_ap = ps[:, T:2 * T]
                        else:
                            out_ap = ps[:, :]
                        nc.tensor.matmul(
                            out_ap, ap(sref), ap(mref),
                            start=(u == 0), stop=(u == L - 1),
                        )
                    if lane % CH == 0:
                        o_t = o_pool.tile([P, CH, T], dt_out, tag="o",
                                          name=f"o_{lane // CH}")
                    m = lane % CH
                    if wide and not ch["cross"]:
                        if ci % 2 == 0:
                            nc.vector.tensor_copy(o_t[:, m:m + 2, :], ps[:, :])
                        else:
                            nc.scalar.copy(o_t[:, m:m + 2, :], ps[:, :])
                    elif wide:  # cross: right half -> first slot, left -> 2nd
                        nc.vector.tensor_copy(o_t[:, m, :], ps[:, T:2 * T])
                        nc.scalar.copy(o_t[:, m + 1, :], ps[:, 0:T])
                    else:
                        if ci % 2 == 0:
                            nc.vector.tensor_copy(o_t[:, m, :], ps[:, :])
                        else:
                            nc.scalar.copy(o_t[:, m, :], ps[:, :])
                    lane += ch["nout"]
                    if lane % CH == 0:
                        nc.gpsimd.dma_start(out=c_out[lane // CH - 1],
                                            in_=o_t)
                        flushed = lane
                    elif ci == len(TEMPLATE) - 1:
                        nl = lane - flushed
                        nc.gpsimd.dma_start(
                            out=c_out[lane // CH][:, :nl, :],
                            in_=o_t[:, :nl, :])
    nc.finalize()
    return nc


def _get_program(repeat=1):
    if repeat not in _PROGRAMS:
        _PROGRAMS[repeat] = _build_program(repeat)
    return _PROGRAMS[repeat]


def _build_in_maps(A, B):
    tri_mask = np.triu(np.ones((T, T), dtype=np.float32))
    np_in = np.float16 if INPUT_DTYPE == "float16" else np.float32
    cache = {}

    def get_block(blk):
        if blk is None:
            return np.zeros((T, T), dtype=np_in)
        mat, bi, bj = blk
        key = (mat, bi, bj)
        if key not in cache:
            M = A if mat == "A" else B
            b = M[bi * T:(bi + 1) * T, bj * T:(bj + 1) * T]
            if bi == bj:
                b = b * tri_mask
            packed = np.ascontiguousarray(b.T) if mat == "A" else b
            cache[key] = packed.astype(np_in)
        return cache[key]

    in_maps = []
    for c in range(NCORES):
        fill, _ = _FILLINGS[c]
        m = {}
        for stack, size in [("pan", NPAN), ("tri", NTRI_PAD),
                            ("ex", NEX_PAD)]:
            arr = np.zeros((size, P, T), dtype=np_in)
            for idx, blk in fill[stack].items():
                arr[idx] = get_block(blk)
            m[stack] = np.ascontiguousarray(
                arr.reshape(size // CH, CH, P, T).transpose(0, 2, 1, 3))
        for stack, nlanes in [("trip", NTRIP_LANES), ("exp", NEXP_LANES)]:
            arr = np.zeros((nlanes, P, T), dtype=np_in)
            for idx, (b0, b1) in fill[stack].items():
                arr[2 * idx] = get_block(b0)
                arr[2 * idx + 1] = get_block(b1)
            m[stack] = np.ascontiguousarray(
                arr.reshape(nlanes // CH, CH, P, T).transpose(0, 2, 1, 3))
        in_maps.append(m)
    return in_maps


def _unpack(results):
    C = np.zeros((N, N), dtype=np.float32)
    for c in range(NCORES):
        out = results[c]["out_stack"].astype(np.float32)
        out = out.transpose(0, 2, 1, 3).reshape(NSLOTS_PAD, P, T)
        _, outs = _FILLINGS[c]
        for s, (oi, oj, transposed) in enumerate(outs):
            part = out[s]
            if transposed:
                part = part.T
            C[oi * T:(oi + 1) * T, oj * T:(oj + 1) * T] += part
    return C


def _emulate(A, B):
    in_maps = _build_in_maps(A, B)
    results = []
    for c in range(NCORES):
        m = in_maps[c]

        def lanes(stack, idx):
            ch_, l_ = idx // CH, idx % CH
            return m[stack][ch_][:, l_, :].astype(np.float32)

        def mov(ref):
            stack, idx = ref
            if stack in ("trip", "exp", "tripL", "expR"):
                base = "trip" if stack.startswith("trip") else "exp"
                l0 = 2 * idx
                if stack == "tripL":
                    return lanes(base, l0)
                if stack == "expR":
                    return lanes(base, l0 + 1)
                return np.concatenate([lanes(base, l0), lanes(base, l0 + 1)],
                                      axis=1)
            return lanes(stack, idx)

        out = np.zeros((NSLOTS_PAD, P, T), dtype=np.float32)
        for ch in TEMPLATE:
            wide = ch["nout"] == 2
            ps = np.zeros((P, 2 * T if wide else T), dtype=np.float32)
            for sref, mref in ch["units"]:
                s = mov(sref)
                mv = mov(mref)
                if mref[0] == "tripL":
                    ps[:, 0:T] += s.T @ mv
                elif mref[0] == "expR":
                    ps[:, T:2 * T] += s.T @ mv
                else:
                    ps += s.T @ mv
            if wide and ch["cross"]:
                out[ch["out"]] = ps[:, T:2 * T]
                out[ch["out"] + 1] = ps[:, 0:T]
            elif wide:
                out[ch["out"]] = ps[:, 0:T]
                out[ch["out"] + 1] = ps[:, T:2 * T]
            else:
                out[ch["out"]] = ps
        results.append({"out_stack": np.ascontiguousarray(
            out.reshape(NSLOTS_PAD // CH, CH, P, T).transpose(0, 2, 1, 3))})
    return _unpack(results)


def kernel(A, B):
    from concourse.bass_utils import run_bass_kernel_spmd

    A = np.asarray(A, dtype=np.float32)
    B = np.asarray(B, dtype=np.float32)
    nc = _get_program()
    in_maps = _build_in_maps(A, B)
    res = run_bass_kernel_spmd(nc, in_maps, list(range(NCORES)))
    return _unpack(res.results)


if __name__ == "__main__":
    rng = np.random.default_rng(0)
    A = rng.standard_normal((N, N), dtype=np.float32)
    B = rng.standard_normal((N, N), dtype=np.float32)
    ref = np.triu(np.triu(A).astype(np.float32) @ np.triu(B))
    got = _emulate(A, B)
    rel = np.linalg.norm(got - ref) / np.linalg.norm(ref)
    print(f"emulation rel err: {rel:.3e}")
    assert rel < 2e-3, rel
    print("emulation OK")


# revision 9
# speedup vs baseline: 1.1756x; 1.1756x over previous
"""Triangular GEMM C = triu(A)@triu(B), N=4096 fp32, 8 trn2 cores, T=128.

Baseline ladder decomposition (126 chains, 748 matmuls/core, 128-wide
moving) upgraded with PAIRED ladders: adjacent output rows of the col-L16,
col-L8 and row-L8 ladders merge into one chain with 256-wide moving
operands (two A-rows / two B-cols packed side by side), halving the matmul
count for ~92% of the work. Stationary per k is shared by the pair by
construction. Per-core matmuls: 748 -> 432.

Uniform SPMD program; per-core behavior via host packing (A-cores 0-3,
B-cores 4-7 = transpose image, same as baseline).
"""

import numpy as np

N = 4096
T = 128
NB = N // T  # 32
P = 128
NCORES = 8

INPUT_DTYPE = "float16"
OUT_DTYPE = "float16"
PSUM_BUFS = 8
NPAN = 64
NSLOTS = 126
CH = 8
NSLOTS_PAD = 128
O_BUFS = 3

# ---- ex stack layout (row-L8 mov block removed vs baseline) ----
E_CL8S = 0      # 2x8  col-L8 stats
E_RL8S = 16     # 2x8  row-L8 stats (reversed)
E_CL4S0 = 32    # 4    col-L4 ss0 stats
E_CL4S1 = 36    # 4    col-L4 ss1 stats
E_RL4S0 = 40    # 4    row-L4 ss0 stats (reversed)
E_RL4M0 = 44    # 10   row-L4 ss0 movs
E_RL4S1 = 54    # 4    row-L4 ss1 stats (reversed)
E_RL4M1 = 58    # 10   row-L4 ss1 movs
E_CL2 = 68      # 2x(2+3) col-L2 a/b stats+movs
E_RL2 = 78      # 2x(2+3) row-L2 a/b stats+movs
E_S2 = 88       # 2x6  s2 cleanup
NEX = 100
NEX_PAD = 104

# tri stack: only the 20 entries col-L4 movs need (triu[0,4) + triu[8,12))
_TRI_LIST = ([(r, s) for r in range(4) for s in range(r, 4)]
             + [(8 + r, 8 + s) for r in range(4) for s in range(r, 4)])
TRI_MAP = {rs: i for i, rs in enumerate(_TRI_LIST)}
NTRI = 20
NTRI_PAD = 24

# trip stack: paired A-rows (2p, 2p+1) x k, k >= 2p: entries [P, 2T]
_TRIP_LIST = [(p, k) for p in range(8) for k in range(2 * p, 16)]
TRIP_MAP = {pk: i for i, pk in enumerate(_TRIP_LIST)}
NTRIP = len(_TRIP_LIST)              # 72 entries = 144 lanes
NTRIP_LANES = 144

# exp stack: paired B-cols for row-L8: entry (r, cp): B(8+r, 8+2cp:8+2cp+2),
# left half zero when 2cp < r (below diagonal)
_EXP_LIST = [(r, cp) for cp in range(4) for r in range(2 * cp + 2)]
EXP_MAP = {rc: i for i, rc in enumerate(_EXP_LIST)}
NEXP = len(_EXP_LIST)                # 20 entries = 40 lanes
NEXP_LANES = 48                      # pad to 6 chunks


def tri4(r, c):
    return r * 4 - r * (r - 1) // 2 + (c - r)


def _build_template():
    """chains: units=[(sref, mref)], mref may be wide; out=first slot,
    nout in {1,2}, cross flag for row-L8 half/lane swap."""
    chains = []
    slot = [0]

    def add(units, nout=1, cross=False):
        chains.append(dict(units=units, out=slot[0], nout=nout, cross=cross))
        slot[0] += nout

    # 4 col-L16 ladders, paired: pair p covers out rows 2p, 2p+1
    for l in range(4):
        for p in range(8):
            units = [(("pan", l * 16 + 2 * p), ("tripL", TRIP_MAP[(p, 2 * p)]))]
            units += [(("pan", l * 16 + k), ("trip", TRIP_MAP[(p, k)]))
                      for k in range(2 * p + 1, 16)]
            add(units, nout=2)
    # 2 col-L8 ladders, paired (k <= 7)
    for l in range(2):
        for p in range(4):
            units = [(("ex", E_CL8S + l * 8 + 2 * p),
                      ("tripL", TRIP_MAP[(p, 2 * p)]))]
            units += [(("ex", E_CL8S + l * 8 + k), ("trip", TRIP_MAP[(p, k)]))
                      for k in range(2 * p + 1, 8)]
            add(units, nout=2)
    # 2 row-L8 ladders, paired: pair q covers out cols 15-2q, 14-2q
    # psum right half = col 15-2q (slot first), left half = 14-2q (second)
    for l in range(2):
        for q in range(4):
            cp = 3 - q
            units = [(("ex", E_RL8S + l * 8 + 2 * q),
                      ("expR", EXP_MAP[(7 - 2 * q, cp)]))]
            units += [(("ex", E_RL8S + l * 8 + 2 * q + u),
                       ("exp", EXP_MAP[(7 - 2 * q - u, cp)]))
                      for u in range(1, 8 - 2 * q)]
            add(units, nout=2, cross=True)
    # col-L4 ss0 / ss1 (unchanged, tri remapped)
    for c in range(4):
        add([(("ex", E_CL4S0 + c + u), ("tri", TRI_MAP[(c, c + u)]))
             for u in range(4 - c)])
    for c in range(4):
        add([(("ex", E_CL4S1 + c + u), ("tri", TRI_MAP[(8 + c, 8 + c + u)]))
             for u in range(4 - c)])
    # row-L4 ss0 / ss1
    for base_s, base_m in ((E_RL4S0, E_RL4M0), (E_RL4S1, E_RL4M1)):
        for c in range(4):
            add([(("ex", base_s + c + u),
                  ("ex", base_m + tri4(3 - c - u, 3 - c)))
                 for u in range(4 - c)])
    # col-L2 a/b
    for g in range(2):
        s = E_CL2 + g * 5
        add([(("ex", s + 0), ("ex", s + 2)), (("ex", s + 1), ("ex", s + 3))])
        add([(("ex", s + 1), ("ex", s + 4))])
    # row-L2 a/b
    for g in range(2):
        s = E_RL2 + g * 5
        add([(("ex", s + 0), ("ex", s + 2)), (("ex", s + 1), ("ex", s + 3))])
        add([(("ex", s + 1), ("ex", s + 4))])
    # s2 x2
    for g in range(2):
        s = E_S2 + g * 6
        add([(("ex", s + 0), ("ex", s + 3)), (("ex", s + 1), ("ex", s + 4))])
        add([(("ex", s + 0), ("ex", s + 5))])
        add([(("ex", s + 2), ("ex", s + 4))])
    assert slot[0] == NSLOTS, slot[0]
    return chains


TEMPLATE = _build_template()


def _acore_filling(c):
    pan, tri, ex, trip, exp = {}, {}, {}, {}, {}
    outs = [None] * NSLOTS

    Jl = [16 + 4 * c + l for l in range(4)]
    for l in range(4):
        for t in range(16):
            pan[l * 16 + t] = ("B", t, Jl[l])
    for (r, s), i in TRI_MAP.items():
        tri[i] = ("A", r, s)
    for (p, k), i in TRIP_MAP.items():
        trip[i] = (("A", 2 * p, k) if k >= 2 * p else None,
                   ("A", 2 * p + 1, k) if k >= 2 * p + 1 else None)
    for (r, cp), i in EXP_MAP.items():
        exp[i] = (("B", 8 + r, 8 + 2 * cp) if 2 * cp >= r else None,
                  ("B", 8 + r, 9 + 2 * cp))
    J8 = [8 + c, 12 + c]
    for l in range(2):
        for t in range(8):
            ex[E_CL8S + l * 8 + t] = ("B", t, J8[l])
    I8 = [2 * c, 2 * c + 1]
    for l in range(2):
        for t in range(8):
            ex[E_RL8S + l * 8 + t] = ("A", I8[l], 15 - t)
    J4a, J4b = 4 + c, 12 + c
    for t in range(4):
        ex[E_CL4S0 + t] = ("B", t, J4a)
        ex[E_CL4S1 + t] = ("B", 8 + t, J4b)
    I4a, I4b = c, 8 + c
    for t in range(4):
        ex[E_RL4S0 + t] = ("A", I4a, 7 - t)
        ex[E_RL4S1 + t] = ("A", I4b, 15 - t)
    for r in range(4):
        for s in range(r, 4):
            ex[E_RL4M0 + tri4(r, s)] = ("B", 4 + r, 4 + s)
            ex[E_RL4M1 + tri4(r, s)] = ("B", 12 + r, 12 + s)
    cl2 = [(4 * c, 4 * c + 2), (4 * ((c + 1) % 4), 4 * ((c + 1) % 4) + 3)]
    for g, (b, J) in enumerate(cl2):
        s = E_CL2 + g * 5
        ex[s + 0] = ("B", b, J)
        ex[s + 1] = ("B", b + 1, J)
        ex[s + 2] = ("A", b, b)
        ex[s + 3] = ("A", b, b + 1)
        ex[s + 4] = ("A", b + 1, b + 1)
    rl2 = [(4 * ((c + 2) % 4), 4 * ((c + 2) % 4)),
           (4 * ((c + 3) % 4), 4 * ((c + 3) % 4) + 1)]
    for g, (b, I) in enumerate(rl2):
        s = E_RL2 + g * 5
        ex[s + 0] = ("A", I, b + 3)
        ex[s + 1] = ("A", I, b + 2)
        ex[s + 2] = ("B", b + 3, b + 3)
        ex[s + 3] = ("B", b + 2, b + 3)
        ex[s + 4] = ("B", b + 2, b + 2)
    b2s = [4 * c, 4 * c + 2]
    for g, b in enumerate(b2s):
        s = E_S2 + g * 6
        ex[s + 0] = ("A", b, b)
        ex[s + 1] = ("A", b, b + 1)
        ex[s + 2] = ("A", b + 1, b + 1)
        ex[s + 3] = ("B", b, b + 1)
        ex[s + 4] = ("B", b + 1, b + 1)
        ex[s + 5] = ("B", b, b)

    # ---- out slots (identical to baseline) ----
    slot = 0
    for l in range(4):
        for cc in range(16):
            outs[slot] = (cc, Jl[l], True); slot += 1
    for l in range(2):
        for cc in range(8):
            outs[slot] = (cc, J8[l], True); slot += 1
    for l in range(2):
        for cc in range(8):
            outs[slot] = (I8[l], 15 - cc, False); slot += 1
    for cc in range(4):
        outs[slot] = (cc, J4a, True); slot += 1
    for cc in range(4):
        outs[slot] = (8 + cc, J4b, True); slot += 1
    for cc in range(4):
        outs[slot] = (I4a, 7 - cc, False); slot += 1
    for cc in range(4):
        outs[slot] = (I4b, 15 - cc, False); slot += 1
    for g, (b, J) in enumerate(cl2):
        outs[slot] = (b, J, True); slot += 1
        outs[slot] = (b + 1, J, True); slot += 1
    for g, (b, I) in enumerate(rl2):
        outs[slot] = (I, b + 3, False); slot += 1
        outs[slot] = (I, b + 2, False); slot += 1
    for g, b in enumerate(b2s):
        outs[slot] = (b, b + 1, False); slot += 1
        outs[slot] = (b, b, False); slot += 1
        outs[slot] = (b + 1, b + 1, False); slot += 1
    assert slot == NSLOTS
    return dict(pan=pan, tri=tri, ex=ex, trip=trip, exp=exp), outs


def _tblock(blk):
    if blk is None:
        return None
    mat, bi, bj = blk
    return (("B", 31 - bj, 31 - bi) if mat == "A" else ("A", 31 - bj, 31 - bi))


def _bcore_filling(c):
    fill, outs = _acore_filling(c - 4)
    tfill = {}
    for stack, mp in fill.items():
        tfill[stack] = {}
        for idx, v in mp.items():
            if stack in ("trip", "exp"):
                tfill[stack][idx] = (_tblock(v[0]), _tblock(v[1]))
            else:
                tfill[stack][idx] = _tblock(v)
    touts = [(31 - J, 31 - I, not tr) for (I, J, tr) in outs]
    return tfill, touts


_FILLINGS = [(_acore_filling(c) if c < 4 else _bcore_filling(c))
             for c in range(NCORES)]


def _check_cover():
    seen = {}
    for c in range(NCORES):
        fill, outs = _FILLINGS[c]

        def get(ref):
            stack, idx = ref[0], ref[1]
            return fill[stack][idx]

        for ch in TEMPLATE:
            slots = [outs[ch["out"] + i] for i in range(ch["nout"])]
            for (ss, si), (ms_, mi) in ch["units"]:
                sb = fill[ss][si]
                if ms_ in ("trip", "exp", "tripL", "expR"):
                    stack = "trip" if ms_.startswith("trip") else "exp"
                    pair = fill[stack][mi]
                    halves = ([pair[0]] if ms_ == "tripL" else
                              [pair[1]] if ms_ == "expR" else list(pair))
                    if ms_ == "tripL":
                        outsl = [slots[0]]
                    elif ms_ == "expR":
                        outsl = [slots[0]]
                    elif ch["cross"]:
                        outsl = [slots[1], slots[0]]
                    else:
                        outsl = slots
                else:
                    halves = [fill[ms_][mi]]
                    outsl = [slots[0]]
                for mb, (I, J, transposed) in zip(halves, outsl):
                    if mb is None:
                        continue
                    ab = sb if sb[0] == "A" else mb
                    bb = sb if sb[0] == "B" else mb
                    assert ab[0] == "A" and bb[0] == "B", (c, sb, mb)
                    assert ab[1] == I and bb[2] == J, (c, I, J, ab, bb)
                    K = ab[2]
                    assert bb[1] == K, (c, I, J, K, ab, bb)
                    assert I <= K <= J, (c, I, K, J)
                    key = (I, K, J)
                    assert key not in seen, (key, seen.get(key), c)
                    seen[key] = c
    want = {(i, k, j) for i in range(NB) for k in range(i, NB)
            for j in range(k, NB)}
    assert set(seen) == want, (len(seen), len(want))


_check_cover()

_PROGRAMS = {}


def _build_program(repeat=1):
    import contextlib
    import concourse.bacc as bacc
    import concourse.mybir as mybir
    from concourse.tile import TileContext

    dt_in = getattr(mybir.dt, INPUT_DTYPE)
    dt_out = getattr(mybir.dt, OUT_DTYPE)
    f32 = mybir.dt.float32
    nc = bacc.Bacc("TRN2", target_bir_lowering=False, debug=False,
                   num_devices=NCORES)
    pan_in = nc.dram_tensor("pan", [NPAN // CH, P, CH, T], dt_in,
                            kind="ExternalInput")
    tri_in = nc.dram_tensor("tri", [NTRI_PAD // CH, P, CH, T], dt_in,
                            kind="ExternalInput")
    ex_in = nc.dram_tensor("ex", [NEX_PAD // CH, P, CH, T], dt_in,
                           kind="ExternalInput")
    trip_in = nc.dram_tensor("trip", [NTRIP_LANES // CH, P, CH * T], dt_in,
                             kind="ExternalInput")
    exp_in = nc.dram_tensor("exp", [NEXP_LANES // CH, P, CH * T], dt_in,
                            kind="ExternalInput")
    c_out = nc.dram_tensor("out_stack", [NSLOTS_PAD // CH, P, CH, T], dt_out,
                           kind="ExternalOutput")

    nch = {"pan": NPAN // CH, "tri": NTRI_PAD // CH, "ex": NEX_PAD // CH,
           "trip": NTRIP_LANES // CH, "exp": NEXP_LANES // CH}
    srcs = {"pan": pan_in, "tri": tri_in, "ex": ex_in, "trip": trip_in,
            "exp": exp_in}

    with TileContext(nc) as tc:
        with (
            tc.tile_pool(name="pan_pool", bufs=2 * nch["pan"]) as pan_pool,
            tc.tile_pool(name="tri_pool", bufs=2 * nch["tri"]) as tri_pool,
            tc.tile_pool(name="ex_pool", bufs=2 * nch["ex"]) as ex_pool,
            tc.tile_pool(name="trip_pool", bufs=2 * nch["trip"]) as trip_pool,
            tc.tile_pool(name="exp_pool", bufs=2 * nch["exp"]) as exp_pool,
            tc.tile_pool(name="o_pool", bufs=O_BUFS) as o_pool,
            tc.tile_pool(name="psum", bufs=PSUM_BUFS, space="PSUM") as psum_pool,
        ):
            pools = {"pan": pan_pool, "tri": tri_pool, "ex": ex_pool,
                     "trip": trip_pool, "exp": exp_pool}
            loop_ctx = (tc.For_i(0, repeat, 1) if repeat > 1
                        else contextlib.nullcontext())
            with loop_ctx:
                chunks = {}

                def load(stack, cidx):
                    shape = ([P, CH * T] if stack in ("trip", "exp")
                             else [P, CH, T])
                    t_ = pools[stack].tile(shape, dt_in, tag=stack,
                                           name=f"{stack}_{cidx}")
                    nc.sync.dma_start(out=t_, in_=srcs[stack][cidx])
                    chunks[(stack, cidx)] = t_

                for t in range(2):
                    load("pan", t)
                for i in range(nch["trip"]):
                    load("trip", i)
                for t in range(2, nch["pan"]):
                    load("pan", t)
                for i in range(nch["exp"]):
                    load("exp", i)
                for i in range(nch["tri"]):
                    load("tri", i)
                for i in range(nch["ex"]):
                    load("ex", i)

                def ap(ref):
                    stack, idx = ref
                    if stack in ("trip", "exp", "tripL", "expR"):
                        base = "trip" if stack.startswith("trip") else "exp"
                        lane = 2 * idx
                        tile = chunks[(base, lane // CH)]
                        o = lane % CH
                        if stack == "tripL":
                            return tile[:, o * T:(o + 1) * T]
                        if stack == "expR":
                            return tile[:, (o + 1) * T:(o + 2) * T]
                        return tile[:, o * T:(o + 2) * T]
                    return chunks[(stack, idx // CH)][:, idx % CH, :]

                o_t = None
                lane = 0
                flushed = 0
                for ci, ch in enumerate(TEMPLATE):
                    units = ch["units"]
                    L = len(units)
                    wide = ch["nout"] == 2
                    ps = psum_pool.tile([P, 2 * T] if wide else [P, T], f32,
                                        tag="ps", name=f"ps_{ch['out']}")
                    for u, (sref, mref) in enumerate(units):
                        if mref[0] in ("tripL",):
                            out_ap = ps[:, 0:T]
                        elif mref[0] in ("expR",):
                            out_ap = ps[:, T:2 * T]
                        else:
                            out_ap = ps[:, :]
                        nc.tensor.matmul(
                            out_ap, ap(sref), ap(mref),
                            start=(u == 0), stop=(u == L - 1),
                        )
                    if lane % CH == 0:
                        o_t = o_pool.tile([P, CH, T], dt_out, tag="o",
                                          name=f"o_{lane // CH}")
                    m = lane % CH
                    if wide and not ch["cross"]:
                        if ci % 2 == 0:
                            nc.vector.tensor_copy(o_t[:, m:m + 2, :], ps[:, :])
                        else:
                            nc.scalar.copy(o_t[:, m:m + 2, :], ps[:, :])
                    elif wide:  # cross: right half -> first slot, left -> 2nd
                        nc.vector.tensor_copy(o_t[:, m, :], ps[:, T:2 * T])
                        nc.scalar.copy(o_t[:, m + 1, :], ps[:, 0:T])
                    else:
                        if ci % 2 == 0:
                            nc.vector.tensor_copy(o_t[:, m, :], ps[:, :])
                        else:
                            nc.scalar.copy(o_t[:, m, :], ps[:, :])
                    lane += ch["nout"]
                    if lane % CH == 0:
                        nc.gpsimd.dma_start(out=c_out[lane // CH - 1],
                                            in_=o_t)
                        flushed = lane
                    elif ci == len(TEMPLATE) - 1:
                        nl = lane - flushed
                        nc.gpsimd.dma_start(
                            out=c_out[lane // CH][:, :nl, :],
                            in_=o_t[:, :nl, :])
    nc.finalize()
    return nc


def _get_program(repeat=1):
    if repeat not in _PROGRAMS:
        _PROGRAMS[repeat] = _build_program(repeat)
    return _PROGRAMS[repeat]


def _build_in_maps(A, B):
    tri_mask = np.triu(np.ones((T, T), dtype=np.float32))
    np_in = np.float16 if INPUT_DTYPE == "float16" else np.float32
    cache = {}

    def get_block(blk):
        if blk is None:
            return np.zeros((T, T), dtype=np_in)
        mat, bi, bj = blk
        key = (mat, bi, bj)
        if key not in cache:
            M = A if mat == "A" else B
            b = M[bi * T:(bi + 1) * T, bj * T:(bj + 1) * T]
            if bi == bj:
                b = b * tri_mask
            packed = np.ascontiguousarray(b.T) if mat == "A" else b
            cache[key] = packed.astype(np_in)
        return cache[key]

    in_maps = []
    for c in range(NCORES):
        fill, _ = _FILLINGS[c]
        m = {}
        for stack, size in [("pan", NPAN), ("tri", NTRI_PAD),
                            ("ex", NEX_PAD)]:
            arr = np.zeros((size, P, T), dtype=np_in)
            for idx, blk in fill[stack].items():
                arr[idx] = get_block(blk)
            m[stack] = np.ascontiguousarray(
                arr.reshape(size // CH, CH, P, T).transpose(0, 2, 1, 3))
        for stack, nlanes in [("trip", NTRIP_LANES), ("exp", NEXP_LANES)]:
            arr = np.zeros((nlanes, P, T), dtype=np_in)
            for idx, (b0, b1) in fill[stack].items():
                arr[2 * idx] = get_block(b0)
                arr[2 * idx + 1] = get_block(b1)
            m[stack] = np.ascontiguousarray(
                arr.reshape(nlanes // CH, CH, P, T).transpose(0, 2, 1, 3)
                .reshape(nlanes // CH, P, CH * T))
        in_maps.append(m)
    return in_maps


def _unpack(results):
    C = np.zeros((N, N), dtype=np.float32)
    for c in range(NCORES):
        out = results[c]["out_stack"].astype(np.float32)
        out = out.transpose(0, 2, 1, 3).reshape(NSLOTS_PAD, P, T)
        _, outs = _FILLINGS[c]
        for s, (oi, oj, transposed) in enumerate(outs):
            part = out[s]
            if transposed:
                part = part.T
            C[oi * T:(oi + 1) * T, oj * T:(oj + 1) * T] += part
    return C


def _emulate(A, B):
    in_maps = _build_in_maps(A, B)
    results = []
    for c in range(NCORES):
        m = in_maps[c]

        def lanes(stack, idx):
            ch_, l_ = idx // CH, idx % CH
            return m[stack][ch_][:, l_, :].astype(np.float32)

        def flat(stack, lane, w):
            ch_, l_ = lane // CH, lane % CH
            return (m[stack][ch_][:, l_ * T:(l_ + w) * T]
                    .astype(np.float32))

        def mov(ref):
            stack, idx = ref
            if stack in ("trip", "exp", "tripL", "expR"):
                base = "trip" if stack.startswith("trip") else "exp"
                l0 = 2 * idx
                if stack == "tripL":
                    return flat(base, l0, 1)
                if stack == "expR":
                    return flat(base, l0 + 1, 1)
                return flat(base, l0, 2)
            return lanes(stack, idx)

        out = np.zeros((NSLOTS_PAD, P, T), dtype=np.float32)
        for ch in TEMPLATE:
            wide = ch["nout"] == 2
            ps = np.zeros((P, 2 * T if wide else T), dtype=np.float32)
            for sref, mref in ch["units"]:
                s = mov(sref)
                mv = mov(mref)
                if mref[0] == "tripL":
                    ps[:, 0:T] += s.T @ mv
                elif mref[0] == "expR":
                    ps[:, T:2 * T] += s.T @ mv
                else:
                    ps += s.T @ mv
            if wide and ch["cross"]:
                out[ch["out"]] = ps[:, T:2 * T]
                out[ch["out"] + 1] = ps[:, 0:T]
            elif wide:
                out[ch["out"]] = ps[:, 0:T]
                out[ch["out"] + 1] = ps[:, T:2 * T]
            else:
                out[ch["out"]] = ps
        results.append({"out_stack": np.ascontiguousarray(
            out.reshape(NSLOTS_PAD // CH, CH, P, T).transpose(0, 2, 1, 3))})
    return _unpack(results)


def kernel(A, B):
    from concourse.bass_utils import run_bass_kernel_spmd

    A = np.asarray(A, dtype=np.float32)
    B = np.asarray(B, dtype=np.float32)
    nc = _get_program()
    in_maps = _build_in_maps(A, B)
    res = run_bass_kernel_spmd(nc, in_maps, list(range(NCORES)))
    return _unpack(res.results)


if __name__ == "__main__":
    rng = np.random.default_rng(0)
    A = rng.standard_normal((N, N), dtype=np.float32)
    B = rng.standard_normal((N, N), dtype=np.float32)
    ref = np.triu(np.triu(A).astype(np.float32) @ np.triu(B))
    got = _emulate(A, B)
    rel = np.linalg.norm(got - ref) / np.linalg.norm(ref)
    print(f"emulation rel err: {rel:.3e}")
    assert rel < 2e-3, rel
    print("emulation OK")


# revision 10
# speedup vs baseline: 1.3233x; 1.1256x over previous
"""Triangular GEMM C = triu(A)@triu(B), N=4096 fp32, 8 trn2 cores, T=128.

Baseline ladder decomposition (126 chains, 748 matmuls/core, 128-wide
moving) upgraded with PAIRED ladders: adjacent output rows of the col-L16,
col-L8 and row-L8 ladders merge into one chain with 256-wide moving
operands (two A-rows / two B-cols packed side by side), halving the matmul
count for ~92% of the work. Stationary per k is shared by the pair by
construction. Per-core matmuls: 748 -> 432.

Uniform SPMD program; per-core behavior via host packing (A-cores 0-3,
B-cores 4-7 = transpose image, same as baseline).
"""

import numpy as np

N = 4096
T = 128
NB = N // T  # 32
P = 128
NCORES = 8

INPUT_DTYPE = "float16"
OUT_DTYPE = "float16"
PSUM_BUFS = 8
NPAN = 64
NSLOTS = 126
CH = 8
NSLOTS_PAD = 128
O_BUFS = 3

# ---- ex stack layout (row-L8 mov block removed vs baseline) ----
E_CL8S = 0      # 2x8  col-L8 stats
E_RL8S = 16     # 2x8  row-L8 stats (reversed)
E_CL4S0 = 32    # 4    col-L4 ss0 stats
E_CL4S1 = 36    # 4    col-L4 ss1 stats
E_RL4S0 = 40    # 4    row-L4 ss0 stats (reversed)
E_RL4M0 = 44    # 10   row-L4 ss0 movs
E_RL4S1 = 54    # 4    row-L4 ss1 stats (reversed)
E_RL4M1 = 58    # 10   row-L4 ss1 movs
E_CL2 = 68      # 2x(2+3) col-L2 a/b stats+movs
E_RL2 = 78      # 2x(2+3) row-L2 a/b stats+movs
E_S2 = 88       # 2x6  s2 cleanup
NEX = 100
NEX_PAD = 104

# tri stack: only the 20 entries col-L4 movs need (triu[0,4) + triu[8,12))
_TRI_LIST = ([(r, s) for r in range(4) for s in range(r, 4)]
             + [(8 + r, 8 + s) for r in range(4) for s in range(r, 4)])
TRI_MAP = {rs: i for i, rs in enumerate(_TRI_LIST)}
NTRI = 20
NTRI_PAD = 24

# trip stack: paired A-rows (2p, 2p+1) x k, k >= 2p: entries [P, 2T]
_TRIP_LIST = [(p, k) for p in range(8) for k in range(2 * p, 16)]
TRIP_MAP = {pk: i for i, pk in enumerate(_TRIP_LIST)}
NTRIP = len(_TRIP_LIST)              # 72 entries = 144 lanes
NTRIP_LANES = 144

# exp stack: paired B-cols for row-L8: entry (r, cp): B(8+r, 8+2cp:8+2cp+2),
# left half zero when 2cp < r (below diagonal)
_EXP_LIST = [(r, cp) for cp in range(4) for r in range(2 * cp + 2)]
EXP_MAP = {rc: i for i, rc in enumerate(_EXP_LIST)}
NEXP = len(_EXP_LIST)                # 20 entries = 40 lanes
NEXP_LANES = 48                      # pad to 6 chunks


def tri4(r, c):
    return r * 4 - r * (r - 1) // 2 + (c - r)


def _build_template():
    """chains: units=[(sref, mref)], mref may be wide; out=first slot,
    nout in {1,2}, cross flag for row-L8 half/lane swap."""
    chains = []
    slot = [0]

    def add(units, nout=1, cross=False):
        chains.append(dict(units=units, out=slot[0], nout=nout, cross=cross))
        slot[0] += nout

    # 4 col-L16 ladders, paired: pair p covers out rows 2p, 2p+1
    for l in range(4):
        for p in range(8):
            units = [(("pan", l * 16 + 2 * p), ("tripL", TRIP_MAP[(p, 2 * p)]))]
            units += [(("pan", l * 16 + k), ("trip", TRIP_MAP[(p, k)]))
                      for k in range(2 * p + 1, 16)]
            add(units, nout=2)
    # 2 col-L8 ladders, paired (k <= 7)
    for l in range(2):
        for p in range(4):
            units = [(("ex", E_CL8S + l * 8 + 2 * p),
                      ("tripL", TRIP_MAP[(p, 2 * p)]))]
            units += [(("ex", E_CL8S + l * 8 + k), ("trip", TRIP_MAP[(p, k)]))
                      for k in range(2 * p + 1, 8)]
            add(units, nout=2)
    # 2 row-L8 ladders, paired: pair q covers out cols 15-2q, 14-2q
    # psum right half = col 15-2q (slot first), left half = 14-2q (second)
    for l in range(2):
        for q in range(4):
            cp = 3 - q
            units = [(("ex", E_RL8S + l * 8 + 2 * q),
                      ("expR", EXP_MAP[(7 - 2 * q, cp)]))]
            units += [(("ex", E_RL8S + l * 8 + 2 * q + u),
                       ("exp", EXP_MAP[(7 - 2 * q - u, cp)]))
                      for u in range(1, 8 - 2 * q)]
            add(units, nout=2, cross=True)
    # col-L4 ss0 / ss1 (unchanged, tri remapped)
    for c in range(4):
        add([(("ex", E_CL4S0 + c + u), ("tri", TRI_MAP[(c, c + u)]))
             for u in range(4 - c)])
    for c in range(4):
        add([(("ex", E_CL4S1 + c + u), ("tri", TRI_MAP[(8 + c, 8 + c + u)]))
             for u in range(4 - c)])
    # row-L4 ss0 / ss1
    for base_s, base_m in ((E_RL4S0, E_RL4M0), (E_RL4S1, E_RL4M1)):
        for c in range(4):
            add([(("ex", base_s + c + u),
                  ("ex", base_m + tri4(3 - c - u, 3 - c)))
                 for u in range(4 - c)])
    # col-L2 a/b
    for g in range(2):
        s = E_CL2 + g * 5
        add([(("ex", s + 0), ("ex", s + 2)), (("ex", s + 1), ("ex", s + 3))])
        add([(("ex", s + 1), ("ex", s + 4))])
    # row-L2 a/b
    for g in range(2):
        s = E_RL2 + g * 5
        add([(("ex", s + 0), ("ex", s + 2)), (("ex", s + 1), ("ex", s + 3))])
        add([(("ex", s + 1), ("ex", s + 4))])
    # s2 x2
    for g in range(2):
        s = E_S2 + g * 6
        add([(("ex", s + 0), ("ex", s + 3)), (("ex", s + 1), ("ex", s + 4))])
        add([(("ex", s + 0), ("ex", s + 5))])
        add([(("ex", s + 2), ("ex", s + 4))])
    assert slot[0] == NSLOTS, slot[0]
    return chains


TEMPLATE = _build_template()


def _acore_filling(c):
    pan, tri, ex, trip, exp = {}, {}, {}, {}, {}
    outs = [None] * NSLOTS

    Jl = [16 + 4 * c + l for l in range(4)]
    for l in range(4):
        for t in range(16):
            pan[l * 16 + t] = ("B", t, Jl[l])
    for (r, s), i in TRI_MAP.items():
        tri[i] = ("A", r, s)
    for (p, k), i in TRIP_MAP.items():
        trip[i] = (("A", 2 * p, k) if k >= 2 * p else None,
                   ("A", 2 * p + 1, k) if k >= 2 * p + 1 else None)
    for (r, cp), i in EXP_MAP.items():
        exp[i] = (("B", 8 + r, 8 + 2 * cp) if 2 * cp >= r else None,
                  ("B", 8 + r, 9 + 2 * cp))
    J8 = [8 + c, 12 + c]
    for l in range(2):
        for t in range(8):
            ex[E_CL8S + l * 8 + t] = ("B", t, J8[l])
    I8 = [2 * c, 2 * c + 1]
    for l in range(2):
        for t in range(8):
            ex[E_RL8S + l * 8 + t] = ("A", I8[l], 15 - t)
    J4a, J4b = 4 + c, 12 + c
    for t in range(4):
        ex[E_CL4S0 + t] = ("B", t, J4a)
        ex[E_CL4S1 + t] = ("B", 8 + t, J4b)
    I4a, I4b = c, 8 + c
    for t in range(4):
        ex[E_RL4S0 + t] = ("A", I4a, 7 - t)
        ex[E_RL4S1 + t] = ("A", I4b, 15 - t)
    for r in range(4):
        for s in range(r, 4):
            ex[E_RL4M0 + tri4(r, s)] = ("B", 4 + r, 4 + s)
            ex[E_RL4M1 + tri4(r, s)] = ("B", 12 + r, 12 + s)
    cl2 = [(4 * c, 4 * c + 2), (4 * ((c + 1) % 4), 4 * ((c + 1) % 4) + 3)]
    for g, (b, J) in enumerate(cl2):
        s = E_CL2 + g * 5
        ex[s + 0] = ("B", b, J)
        ex[s + 1] = ("B", b + 1, J)
        ex[s + 2] = ("A", b, b)
        ex[s + 3] = ("A", b, b + 1)
        ex[s + 4] = ("A", b + 1, b + 1)
    rl2 = [(4 * ((c + 2) % 4), 4 * ((c + 2) % 4)),
           (4 * ((c + 3) % 4), 4 * ((c + 3) % 4) + 1)]
    for g, (b, I) in enumerate(rl2):
        s = E_RL2 + g * 5
        ex[s + 0] = ("A", I, b + 3)
        ex[s + 1] = ("A", I, b + 2)
        ex[s + 2] = ("B", b + 3, b + 3)
        ex[s + 3] = ("B", b + 2, b + 3)
        ex[s + 4] = ("B", b + 2, b + 2)
    b2s = [4 * c, 4 * c + 2]
    for g, b in enumerate(b2s):
        s = E_S2 + g * 6
        ex[s + 0] = ("A", b, b)
        ex[s + 1] = ("A", b, b + 1)
        ex[s + 2] = ("A", b + 1, b + 1)
        ex[s + 3] = ("B", b, b + 1)
        ex[s + 4] = ("B", b + 1, b + 1)
        ex[s + 5] = ("B", b, b)

    # ---- out slots (identical to baseline) ----
    slot = 0
    for l in range(4):
        for cc in range(16):
            outs[slot] = (cc, Jl[l], True); slot += 1
    for l in range(2):
        for cc in range(8):
            outs[slot] = (cc, J8[l], True); slot += 1
    for l in range(2):
        for cc in range(8):
            outs[slot] = (I8[l], 15 - cc, False); slot += 1
    for cc in range(4):
        outs[slot] = (cc, J4a, True); slot += 1
    for cc in range(4):
        outs[slot] = (8 + cc, J4b, True); slot += 1
    for cc in range(4):
        outs[slot] = (I4a, 7 - cc, False); slot += 1
    for cc in range(4):
        outs[slot] = (I4b, 15 - cc, False); slot += 1
    for g, (b, J) in enumerate(cl2):
        outs[slot] = (b, J, True); slot += 1
        outs[slot] = (b + 1, J, True); slot += 1
    for g, (b, I) in enumerate(rl2):
        outs[slot] = (I, b + 3, False); slot += 1
        outs[slot] = (I, b + 2, False); slot += 1
    for g, b in enumerate(b2s):
        outs[slot] = (b, b + 1, False); slot += 1
        outs[slot] = (b, b, False); slot += 1
        outs[slot] = (b + 1, b + 1, False); slot += 1
    assert slot == NSLOTS
    return dict(pan=pan, tri=tri, ex=ex, trip=trip, exp=exp), outs


def _tblock(blk):
    if blk is None:
        return None
    mat, bi, bj = blk
    return (("B", 31 - bj, 31 - bi) if mat == "A" else ("A", 31 - bj, 31 - bi))


def _bcore_filling(c):
    fill, outs = _acore_filling(c - 4)
    tfill = {}
    for stack, mp in fill.items():
        tfill[stack] = {}
        for idx, v in mp.items():
            if stack in ("trip", "exp"):
                tfill[stack][idx] = (_tblock(v[0]), _tblock(v[1]))
            else:
                tfill[stack][idx] = _tblock(v)
    touts = [(31 - J, 31 - I, not tr) for (I, J, tr) in outs]
    return tfill, touts


_FILLINGS = [(_acore_filling(c) if c < 4 else _bcore_filling(c))
             for c in range(NCORES)]


def _check_cover():
    seen = {}
    for c in range(NCORES):
        fill, outs = _FILLINGS[c]

        def get(ref):
            stack, idx = ref[0], ref[1]
            return fill[stack][idx]

        for ch in TEMPLATE:
            slots = [outs[ch["out"] + i] for i in range(ch["nout"])]
            for (ss, si), (ms_, mi) in ch["units"]:
                sb = fill[ss][si]
                if ms_ in ("trip", "exp", "tripL", "expR"):
                    stack = "trip" if ms_.startswith("trip") else "exp"
                    pair = fill[stack][mi]
                    halves = ([pair[0]] if ms_ == "tripL" else
                              [pair[1]] if ms_ == "expR" else list(pair))
                    if ms_ == "tripL":
                        outsl = [slots[0]]
                    elif ms_ == "expR":
                        outsl = [slots[0]]
                    elif ch["cross"]:
                        outsl = [slots[1], slots[0]]
                    else:
                        outsl = slots
                else:
                    halves = [fill[ms_][mi]]
                    outsl = [slots[0]]
                for mb, (I, J, transposed) in zip(halves, outsl):
                    if mb is None:
                        continue
                    ab = sb if sb[0] == "A" else mb
                    bb = sb if sb[0] == "B" else mb
                    assert ab[0] == "A" and bb[0] == "B", (c, sb, mb)
                    assert ab[1] == I and bb[2] == J, (c, I, J, ab, bb)
                    K = ab[2]
                    assert bb[1] == K, (c, I, J, K, ab, bb)
                    assert I <= K <= J, (c, I, K, J)
                    key = (I, K, J)
                    assert key not in seen, (key, seen.get(key), c)
                    seen[key] = c
    want = {(i, k, j) for i in range(NB) for k in range(i, NB)
            for j in range(k, NB)}
    assert set(seen) == want, (len(seen), len(want))


_check_cover()

_PROGRAMS = {}


def _build_program(repeat=1):
    import contextlib
    import concourse.bacc as bacc
    import concourse.mybir as mybir
    from concourse.tile import TileContext

    dt_in = getattr(mybir.dt, INPUT_DTYPE)
    dt_out = getattr(mybir.dt, OUT_DTYPE)
    f32 = mybir.dt.float32
    nc = bacc.Bacc("TRN2", target_bir_lowering=False, debug=False,
                   num_devices=NCORES)
    pan_in = nc.dram_tensor("pan", [NPAN // CH, P, CH, T], dt_in,
                            kind="ExternalInput")
    tri_in = nc.dram_tensor("tri", [NTRI_PAD // CH, P, CH, T], dt_in,
                            kind="ExternalInput")
    ex_in = nc.dram_tensor("ex", [NEX_PAD // CH, P, CH, T], dt_in,
                           kind="ExternalInput")
    trip_in = nc.dram_tensor("trip", [NTRIP_LANES // CH, P, CH * T], dt_in,
                             kind="ExternalInput")
    exp_in = nc.dram_tensor("exp", [NEXP_LANES // CH, P, CH * T], dt_in,
                            kind="ExternalInput")
    c_out = nc.dram_tensor("out_stack", [NSLOTS_PAD // CH, P, CH, T], dt_out,
                           kind="ExternalOutput")

    nch = {"pan": NPAN // CH, "tri": NTRI_PAD // CH, "ex": NEX_PAD // CH,
           "trip": NTRIP_LANES // CH, "exp": NEXP_LANES // CH}
    srcs = {"pan": pan_in, "tri": tri_in, "ex": ex_in, "trip": trip_in,
            "exp": exp_in}

    with TileContext(nc) as tc:
        with (
            tc.tile_pool(name="pan_pool", bufs=2 * nch["pan"]) as pan_pool,
            tc.tile_pool(name="tri_pool", bufs=2 * nch["tri"]) as tri_pool,
            tc.tile_pool(name="ex_pool", bufs=2 * nch["ex"]) as ex_pool,
            tc.tile_pool(name="trip_pool", bufs=2 * nch["trip"]) as trip_pool,
            tc.tile_pool(name="exp_pool", bufs=2 * nch["exp"]) as exp_pool,
            tc.tile_pool(name="o_pool", bufs=O_BUFS) as o_pool,
            tc.tile_pool(name="psum", bufs=PSUM_BUFS, space="PSUM") as psum_pool,
        ):
            pools = {"pan": pan_pool, "tri": tri_pool, "ex": ex_pool,
                     "trip": trip_pool, "exp": exp_pool}
            # manual unroll: For_i places an all-engine barrier per
            # iteration, serializing input DMA vs compute; emitting U
            # ticks per body lets the 2x-buffered pools overlap tick
            # t+1's DMA with tick t's compute (barrier cost paid 1/U).
            U = 8 if repeat > 1 and repeat % 8 == 0 else 1
            loop_ctx = (tc.For_i(0, repeat // U, 1) if repeat > U
                        else contextlib.nullcontext())
            with loop_ctx:
              for _tick in range(U if repeat > 1 else 1):
                chunks = {}

                def load(stack, cidx):
                    shape = ([P, CH * T] if stack in ("trip", "exp")
                             else [P, CH, T])
                    t_ = pools[stack].tile(shape, dt_in, tag=stack,
                                           name=f"{stack}_{cidx}")
                    nc.sync.dma_start(out=t_, in_=srcs[stack][cidx])
                    chunks[(stack, cidx)] = t_

                for t in range(2):
                    load("pan", t)
                for i in range(nch["trip"]):
                    load("trip", i)
                for t in range(2, nch["pan"]):
                    load("pan", t)
                for i in range(nch["exp"]):
                    load("exp", i)
                for i in range(nch["tri"]):
                    load("tri", i)
                for i in range(nch["ex"]):
                    load("ex", i)

                def ap(ref):
                    stack, idx = ref
                    if stack in ("trip", "exp", "tripL", "expR"):
                        base = "trip" if stack.startswith("trip") else "exp"
                        lane = 2 * idx
                        tile = chunks[(base, lane // CH)]
                        o = lane % CH
                        if stack == "tripL":
                            return tile[:, o * T:(o + 1) * T]
                        if stack == "expR":
                            return tile[:, (o + 1) * T:(o + 2) * T]
                        return tile[:, o * T:(o + 2) * T]
                    return chunks[(stack, idx // CH)][:, idx % CH, :]

                o_t = None
                lane = 0
                flushed = 0
                for ci, ch in enumerate(TEMPLATE):
                    units = ch["units"]
                    L = len(units)
                    wide = ch["nout"] == 2
                    ps = psum_pool.tile([P, 2 * T] if wide else [P, T], f32,
                                        tag="ps", name=f"ps_{ch['out']}")
                    for u, (sref, mref) in enumerate(units):
                        if mref[0] in ("tripL",):
                            out_ap = ps[:, 0:T]
                        elif mref[0] in ("expR",):
                            out_ap = ps[:, T:2 * T]
                        else:
                            out_ap = ps[:, :]
                        nc.tensor.matmul(
                            out_ap, ap(sref), ap(mref),
                            start=(u == 0), stop=(u == L - 1),
                        )
                    if lane % CH == 0:
                        o_t = o_pool.tile([P, CH, T], dt_out, tag="o",
                                          name=f"o_{lane // CH}")
                    m = lane % CH
                    if wide and not ch["cross"]:
                        if ci % 2 == 0:
                            nc.vector.tensor_copy(o_t[:, m:m + 2, :], ps[:, :])
                        else:
                            nc.scalar.copy(o_t[:, m:m + 2, :], ps[:, :])
                    elif wide:  # cross: right half -> first slot, left -> 2nd
                        nc.vector.tensor_copy(o_t[:, m, :], ps[:, T:2 * T])
                        nc.scalar.copy(o_t[:, m + 1, :], ps[:, 0:T])
                    else:
                        if ci % 2 == 0:
                            nc.vector.tensor_copy(o_t[:, m, :], ps[:, :])
                        else:
                            nc.scalar.copy(o_t[:, m, :], ps[:, :])
                    lane += ch["nout"]
                    if lane % CH == 0:
                        nc.gpsimd.dma_start(out=c_out[lane // CH - 1],
                                            in_=o_t)
                        flushed = lane
                    elif ci == len(TEMPLATE) - 1:
                        nl = lane - flushed
                        nc.gpsimd.dma_start(
                            out=c_out[lane // CH][:, :nl, :],
                            in_=o_t[:, :nl, :])
    nc.finalize()
    return nc


def _get_program(repeat=1):
    if repeat not in _PROGRAMS:
        _PROGRAMS[repeat] = _build_program(repeat)
    return _PROGRAMS[repeat]


def _build_in_maps(A, B):
    tri_mask = np.triu(np.ones((T, T), dtype=np.float32))
    np_in = np.float16 if INPUT_DTYPE == "float16" else np.float32
    cache = {}

    def get_block(blk):
        if blk is None:
            return np.zeros((T, T), dtype=np_in)
        mat, bi, bj = blk
        key = (mat, bi, bj)
        if key not in cache:
            M = A if mat == "A" else B
            b = M[bi * T:(bi + 1) * T, bj * T:(bj + 1) * T]
            if bi == bj:
                b = b * tri_mask
            packed = np.ascontiguousarray(b.T) if mat == "A" else b
            cache[key] = packed.astype(np_in)
        return cache[key]

    in_maps = []
    for c in range(NCORES):
        fill, _ = _FILLINGS[c]
        m = {}
        for stack, size in [("pan", NPAN), ("tri", NTRI_PAD),
                            ("ex", NEX_PAD)]:
            arr = np.zeros((size, P, T), dtype=np_in)
            for idx, blk in fill[stack].items():
                arr[idx] = get_block(blk)
            m[stack] = np.ascontiguousarray(
                arr.reshape(size // CH, CH, P, T).transpose(0, 2, 1, 3))
        for stack, nlanes in [("trip", NTRIP_LANES), ("exp", NEXP_LANES)]:
            arr = np.zeros((nlanes, P, T), dtype=np_in)
            for idx, (b0, b1) in fill[stack].items():
                arr[2 * idx] = get_block(b0)
                arr[2 * idx + 1] = get_block(b1)
            m[stack] = np.ascontiguousarray(
                arr.reshape(nlanes // CH, CH, P, T).transpose(0, 2, 1, 3)
                .reshape(nlanes // CH, P, CH * T))
        in_maps.append(m)
    return in_maps


def _unpack(results):
    C = np.zeros((N, N), dtype=np.float32)
    for c in range(NCORES):
        out = results[c]["out_stack"].astype(np.float32)
        out = out.transpose(0, 2, 1, 3).reshape(NSLOTS_PAD, P, T)
        _, outs = _FILLINGS[c]
        for s, (oi, oj, transposed) in enumerate(outs):
            part = out[s]
            if transposed:
                part = part.T
            C[oi * T:(oi + 1) * T, oj * T:(oj + 1) * T] += part
    return C


def _emulate(A, B):
    in_maps = _build_in_maps(A, B)
    results = []
    for c in range(NCORES):
        m = in_maps[c]

        def lanes(stack, idx):
            ch_, l_ = idx // CH, idx % CH
            return m[stack][ch_][:, l_, :].astype(np.float32)

        def flat(stack, lane, w):
            ch_, l_ = lane // CH, lane % CH
            return (m[stack][ch_][:, l_ * T:(l_ + w) * T]
                    .astype(np.float32))

        def mov(ref):
            stack, idx = ref
            if stack in ("trip", "exp", "tripL", "expR"):
                base = "trip" if stack.startswith("trip") else "exp"
                l0 = 2 * idx
                if stack == "tripL":
                    return flat(base, l0, 1)
                if stack == "expR":
                    return flat(base, l0 + 1, 1)
                return flat(base, l0, 2)
            return lanes(stack, idx)

        out = np.zeros((NSLOTS_PAD, P, T), dtype=np.float32)
        for ch in TEMPLATE:
            wide = ch["nout"] == 2
            ps = np.zeros((P, 2 * T if wide else T), dtype=np.float32)
            for sref, mref in ch["units"]:
                s = mov(sref)
                mv = mov(mref)
                if mref[0] == "tripL":
                    ps[:, 0:T] += s.T @ mv
                elif mref[0] == "expR":
                    ps[:, T:2 * T] += s.T @ mv
                else:
                    ps += s.T @ mv
            if wide and ch["cross"]:
                out[ch["out"]] = ps[:, T:2 * T]
                out[ch["out"] + 1] = ps[:, 0:T]
            elif wide:
                out[ch["out"]] = ps[:, 0:T]
                out[ch["out"] + 1] = ps[:, T:2 * T]
            else:
                out[ch["out"]] = ps
        results.append({"out_stack": np.ascontiguousarray(
            out.reshape(NSLOTS_PAD // CH, CH, P, T).transpose(0, 2, 1, 3))})
    return _unpack(results)


def kernel(A, B):
    from concourse.bass_utils import run_bass_kernel_spmd

    A = np.asarray(A, dtype=np.float32)
    B = np.asarray(B, dtype=np.float32)
    nc = _get_program()
    in_maps = _build_in_maps(A, B)
    res = run_bass_kernel_spmd(nc, in_maps, list(range(NCORES)))
    return _unpack(res.results)


if __name__ == "__main__":
    rng = np.random.default_rng(0)
    A = rng.standard_normal((N, N), dtype=np.float32)
    B = rng.standard_normal((N, N), dtype=np.float32)
    ref = np.triu(np.triu(A).astype(np.float32) @ np.triu(B))
    got = _emulate(A, B)
    rel = np.linalg.norm(got - ref) / np.linalg.norm(ref)
    print(f"emulation rel err: {rel:.3e}")
    assert rel < 2e-3, rel
    print("emulation OK")
